# revision 8
# baseline (speedup 1.0000x reference)
"""Trainium2 Bass kernel for nn_ContextCTRNN.

Math: per timestep t, ctx is blended with the token's context embedding via a
norm-gated beta, then out[b,v] = ae_b^T @ W3[v] @ ctx_b.  The bilinear readout
is restructured as one big matmul:

    out[tb, v] = sum_{c,d} W3[v,c,d] * ae[tb,c] * ctx[tb,d]
               = (m @ W2^T)[tb, v],   m[tb, (c,d)] = ae[tb,c]*ctx[tb,d]

with tb = t*B+b (the scan only affects the tiny [B,C] ctx state, so all T*B
rows are batched).  Sharding: vocab (V) split across 8 cores; each core gets
W2^T's shard [C*C, V/8] plus replicated activations, computes out[:, vshard],
and the host concatenates.

Device kernel per core:
  1. sequential ctx scan over T on [B, C] tiles (DVE/ACT)
  2. PE-transpose ctx -> ctxT2 [128, TB] (two stacked copies of ctx^T)
  3. mT[(c,d), tb] = aeT_rep (host-prepped, DMA'd) * ctxT2   (DVE)
  4. out_psum[tb, v] += mT_k^T @ Wt_k  accumulated over 32 k-tiles (PE)
"""

import os
import sys
import types

import numpy as np

import concourse.bass as bass
import concourse.mybir as mybir
from concourse import bacc
from concourse.tile import TileContext
from concourse.bass_utils import run_bass_kernel_spmd
from concourse.masks import make_identity

# Problem constants (hardcoded per harness contract).
C = 64
V = 16000
T = 64
B = 8
N_CORES = 8
VS = V // N_CORES          # 2000 vocab rows per core
TB = T * B                 # 512 batched rows
K = C * C                  # 4096 contraction
NK = K // 128              # 32 k-tiles
NM = TB // 128             # 4 tb-tiles
VT = 500                   # vocab tile (psum bank limit: 500 f32 <= 2KB)
NVH = 2                    # vocab halves in main loop
VH = VS // NVH             # 1000

F32 = mybir.dt.float32
F32R = mybir.dt.float32r
BF16 = mybir.dt.bfloat16

W_MODE = os.environ.get("CTRNN_W_MODE", "bf16")  # f32 | f32r | bf16


def _np_wdt(w_mode):
    if w_mode == "bf16":
        import ml_dtypes

        return np.dtype(ml_dtypes.bfloat16)
    return np.dtype(np.float32)


def _bir_wdt(w_mode):
    if w_mode == "bf16":
        return BF16
    if w_mode == "f32r":
        return F32R
    return F32


def build_nc(w_mode=W_MODE, beta_mult=1.0, beta_power=1.0, n_cores=N_CORES):
    """Build the (single-program, SPMD) bass kernel."""
    w_dt = _bir_wdt(w_mode)

    def mm_cast(ap):
        return ap

    nc = bacc.Bacc("TRN2", target_bir_lowering=False, debug=False,
                   num_devices=n_cores)

    ce_d = nc.dram_tensor("ce", [TB, C], F32, kind="ExternalInput")
    aer_d = nc.dram_tensor("aer", [K, TB], w_dt, kind="ExternalInput")
    wt_d = nc.dram_tensor("wt", [K, VS], w_dt, kind="ExternalInput")
    out_d = nc.dram_tensor("out", [TB, VS], F32, kind="ExternalOutput")

    with TileContext(nc) as tc:
        from contextlib import ExitStack

        with (
            tc.tile_pool(name="const", bufs=1) as const_pool,
            tc.tile_pool(name="scan", bufs=1) as scan_pool,
            tc.tile_pool(name="mstage", bufs=1) as mstage_pool,
            tc.tile_pool(name="dram", bufs=1, space="DRAM") as dram_pool,
            tc.tile_pool(name="io", bufs=3) as io_pool,
            tc.tile_pool(name="wst", bufs=3) as w_pool,
            tc.tile_pool(name="ost", bufs=3) as out_pool,
        ):
            ps_tr_ctx = ExitStack()
            ps_tr_pool = ps_tr_ctx.enter_context(
                tc.tile_pool(name="ps_tr", bufs=2, space="PSUM"))
            # ---- constants ----
            identity = const_pool.tile([128, 128], F32)
            make_identity(nc, identity)

            # ---- phase A: load ce in [b, t, c] layout; precompute ne ----
            ce_b = scan_pool.tile([B, T, C], F32)
            nc.sync.dma_start(out=ce_b, in_=ce_d.ap().rearrange(
                "(t b) c -> b t c", b=B))

            sq_all = scan_pool.tile([B, T * C], F32)
            nc.vector.tensor_mul(sq_all, ce_b.rearrange("b t c -> b (t c)"),
                                 ce_b.rearrange("b t c -> b (t c)"))
            ne2 = scan_pool.tile([B, T, 1], F32)
            nc.vector.tensor_reduce(out=ne2, in_=sq_all.rearrange(
                "b (t c) -> b t c", t=T), axis=mybir.AxisListType.X,
                op=mybir.AluOpType.add)
            ne = scan_pool.tile([B, T], F32)
            nc.scalar.activation(out=ne, in_=ne2.rearrange("b t one -> b (t one)"),
                                 func=mybir.ActivationFunctionType.Sqrt)

            # ---- phase B: sequential scan over T ----
            # ctx_seq[:, 0, :] is the zero initial state; step t writes t+1.
            ctx_seq = scan_pool.tile([B, T + 1, C], F32)
            nc.vector.memset(ctx_seq[:, 0, :], 0.0)
            ncur = scan_pool.tile([B, 1], F32)   # ||ctx_t||
            nc.vector.memset(ncur, 0.0)
            ssum = scan_pool.tile([B, 1], F32)
            rinv = scan_pool.tile([B, 1], F32)
            beta = scan_pool.tile([B, 1], F32)
            nc2 = scan_pool.tile([B, 1], F32)
            dvec = scan_pool.tile([B, C], F32)
            sqv = scan_pool.tile([B, C], F32)

            general_beta = (beta_mult != 1.0) or (beta_power != 1.0)
            for t in range(T):
                ne_t = ne[:, t:t + 1]
                ctx_prev = ctx_seq[:, t, :]
                ctx_t = ctx_seq[:, t + 1, :]
                # beta = beta_mult * ne_t / (ne_t + ||ctx_prev||)
                nc.vector.tensor_add(ssum, ne_t, ncur)
                nc.vector.reciprocal(rinv, ssum)
                if beta_mult == 1.0:
                    nc.vector.tensor_mul(beta, ne_t, rinv)
                else:
                    nc.vector.scalar_tensor_tensor(
                        out=beta, in0=ne_t, scalar=float(beta_mult),
                        in1=rinv, op0=mybir.AluOpType.mult,
                        op1=mybir.AluOpType.mult)
                if beta_power != 1.0:
                    nc.scalar.activation(out=beta, in_=beta,
                                         func=mybir.ActivationFunctionType.Ln)
                    nc.vector.tensor_scalar_mul(beta, beta, float(beta_power))
                    nc.scalar.activation(out=beta, in_=beta,
                                         func=mybir.ActivationFunctionType.Exp)
                if general_beta:
                    nc.vector.tensor_scalar_min(beta, beta, 1.0)
                    nc.vector.tensor_scalar_max(beta, beta, 0.0)
                # ctx_t = ctx_prev + beta * (ce_t - ctx_prev)
                nc.vector.tensor_sub(dvec, ce_b[:, t, :], ctx_prev)
                nc.vector.scalar_tensor_tensor(
                    out=ctx_t, in0=dvec, scalar=beta, in1=ctx_prev,
                    op0=mybir.AluOpType.mult, op1=mybir.AluOpType.add)
                # ||ctx_t||
                nc.vector.scalar_tensor_tensor(
                    out=sqv, in0=ctx_t, scalar=1.0, in1=ctx_t,
                    op0=mybir.AluOpType.mult, op1=mybir.AluOpType.mult,
                    accum_out=nc2)
                nc.scalar.activation(out=ncur, in_=nc2,
                                     func=mybir.ActivationFunctionType.Sqrt)

            # ---- phase C: reshape ctx to (t b) rows via DRAM round-trip ----
            ctx_dram = dram_pool.tile([TB, C], F32)
            nc.sync.dma_start(out=ctx_dram.rearrange("(t b) c -> b t c", b=B),
                              in_=ctx_seq[:, 1:, :])

            # ---- phase D: ctxT2[(j,d), tb] = ctx[tb, d]  (j = 0,1) ----
            ctxT2 = mstage_pool.tile([128, TB], F32)
            for m0 in range(NM):
                ctx_tb = io_pool.tile([128, C], F32, name=f"ctx_tb_{m0}",
                                      tag="ctx_tb")
                nc.sync.dma_start(out=ctx_tb,
                                  in_=ctx_dram[m0 * 128:(m0 + 1) * 128, :])
                ctx_ps = ps_tr_pool.tile([C, 128], F32, name=f"ctx_ps_{m0}",
                                         tag="ctx_ps")
                nc.tensor.transpose(ctx_ps, ctx_tb, identity)
                nc.vector.tensor_copy(out=ctxT2[0:C, m0 * 128:(m0 + 1) * 128],
                                      in_=ctx_ps)
            # replicate ctx^T into the lower 64 partitions (cross-partition
            # moves need DMA, not DVE)
            nc.sync.dma_start(out=ctxT2[C:128, :], in_=ctxT2[0:C, :])

            # ---- phase E: mT[(c,d), tb] = aer * ctxT2 ----
            mT = mstage_pool.tile([128, NK, TB], w_dt)
            for k in range(NK):
                aer_k = io_pool.tile([128, TB], w_dt, name=f"aer_{k}",
                                     tag="aer")
                nc.sync.dma_start(out=aer_k,
                                  in_=aer_d.ap()[k * 128:(k + 1) * 128, :])
                nc.vector.tensor_mul(mT[:, k, :], aer_k, ctxT2)

            # ---- phase F: main matmul, W streamed once ----
            ps_tr_ctx.close()  # release transpose psum banks for accumulators
            ps_acc_ctx = ExitStack()
            ps_acc_pool = ps_acc_ctx.enter_context(
                tc.tile_pool(name="ps_acc", bufs=1, space="PSUM"))
            for vh in range(NVH):
                accs = {}
                for m0 in range(NM):
                    for v2 in range(VH // VT):
                        accs[(m0, v2)] = ps_acc_pool.tile(
                            [128, VT], F32, name=f"acc_{m0}_{v2}",
                            tag=f"acc_{m0}_{v2}")
                for k in range(NK):
                    w_k = w_pool.tile([128, VH], w_dt, name=f"w_{vh}_{k}",
                                      tag="w")
                    nc.sync.dma_start(
                        out=w_k,
                        in_=wt_d.ap()[k * 128:(k + 1) * 128,
                                      vh * VH:(vh + 1) * VH])
                    for m0 in range(NM):
                        lhsT = mm_cast(mT[:, k, m0 * 128:(m0 + 1) * 128])
                        for v2 in range(VH // VT):
                            nc.tensor.matmul(
                                accs[(m0, v2)],
                                lhsT,
                                mm_cast(w_k[:, v2 * VT:(v2 + 1) * VT]),
                                start=(k == 0), stop=(k == NK - 1))
                for m0 in range(NM):
                    for v2 in range(VH // VT):
                        o_sb = out_pool.tile([128, VT], F32,
                                             name=f"o_{vh}_{m0}_{v2}",
                                             tag="o")
                        nc.vector.tensor_copy(out=o_sb, in_=accs[(m0, v2)])
                        col = vh * VH + v2 * VT
                        nc.sync.dma_start(
                            out=out_d.ap()[m0 * 128:(m0 + 1) * 128,
                                           col:col + VT],
                            in_=o_sb)
            ps_acc_ctx.close()

    nc.finalize()
    return nc


def host_prep(tokens, emb_ctx, emb_act, W_a, w_mode=W_MODE):
    """Gather embeddings, build the replicated ae^T and the W shards."""
    wnp = _np_wdt(w_mode)
    tok = np.asarray(tokens).astype(np.int64).reshape(-1)  # [T*B], t-major
    ce = np.ascontiguousarray(np.asarray(emb_ctx, dtype=np.float32)[tok])
    ae = np.asarray(emb_act, dtype=np.float32)[tok]        # [TB, C]
    # aer[(c,d), tb] = ae[tb, c]
    aer = np.ascontiguousarray(
        np.repeat(ae.T, C, axis=0)).astype(wnp)            # [C*C, TB]
    W3 = np.asarray(W_a, dtype=np.float32).reshape(V, C, C)
    in_maps = []
    for s in range(N_CORES):
        ws = np.ascontiguousarray(
            W3[s * VS:(s + 1) * VS].transpose(1, 2, 0).reshape(K, VS)
        ).astype(wnp)                                      # [(c,d), v_local]
        in_maps.append({"ce": ce, "aer": aer, "wt": ws})
    return in_maps


_NC_CACHE = {}


def _get_nc(w_mode, beta_mult, beta_power):
    key = (w_mode, float(beta_mult), float(beta_power))
    if key not in _NC_CACHE:
        _NC_CACHE[key] = build_nc(w_mode, *key[1:])
    return _NC_CACHE[key]


def install_ntff_shim():
    """Optional: register the axon NTFF profiling hook (for tracing)."""
    if "antenv.axon_hooks" in sys.modules:
        return
    m = types.ModuleType("antenv.axon_hooks")
    state = {"hook": None}
    m.get_axon_ntff_profile_hook = lambda: state["hook"]
    m.set_axon_ntff_profile_hook = lambda h: state.update(hook=h)
    sys.modules["antenv.axon_hooks"] = m
    try:
        from trn_agent_boot.trn_boot import _ntff_profile_via_ctypes

        state["hook"] = _ntff_profile_via_ctypes("/opt/axon/libaxon_pjrt.so")
    except Exception:
        pass


def run_hw(tokens, emb_ctx, emb_act, W_a, beta_mult, beta_power,
           w_mode=W_MODE, trace=False):
    if trace:
        install_ntff_shim()
    nc = _get_nc(w_mode, float(beta_mult), float(beta_power))
    in_maps = host_prep(tokens, emb_ctx, emb_act, W_a, w_mode)
    res = run_bass_kernel_spmd(nc, in_maps, core_ids=list(range(N_CORES)),
                               trace=trace)
    out = np.concatenate([res.results[s]["out"] for s in range(N_CORES)],
                         axis=1)                           # [TB, V]
    return out.reshape(T, B, V), res


def kernel(tokens, emb_ctx, emb_act, W_a, beta_mult, beta_power):
    out, _ = run_hw(tokens, emb_ctx, emb_act, W_a,
                    float(np.asarray(beta_mult)), float(np.asarray(beta_power)))
    return out


# revision 10
# speedup vs baseline: 1.2753x; 1.2753x over previous
"""Trainium2 Bass kernel for nn_ContextCTRNN.

Math: per timestep t, ctx is blended with the token's context embedding via a
norm-gated beta, then out[b,v] = ae_b^T @ W3[v] @ ctx_b.  The bilinear readout
is restructured as one big matmul:

    out[tb, v] = sum_{c,d} W3[v,c,d] * ae[tb,c] * ctx[tb,d]
               = (m @ W2^T)[tb, v],   m[tb, (c,d)] = ae[tb,c]*ctx[tb,d]

with tb = t*B+b (the scan only affects the tiny [B,C] ctx state, so all T*B
rows are batched).  Sharding: vocab (V) split across 8 cores; each core gets
W2^T's shard [C*C, V/8] plus replicated activations, computes out[:, vshard],
and the host concatenates.

Device kernel per core:
  1. sequential ctx scan over T on [B, C] tiles (DVE/ACT)
  2. PE-transpose ctx -> ctxT2 [128, TB] (two stacked copies of ctx^T)
  3. mT[(c,d), tb] = aeT_rep (host-prepped, DMA'd) * ctxT2   (DVE)
  4. out_psum[tb, v] += mT_k^T @ Wt_k  accumulated over 32 k-tiles (PE)
"""

import os
import sys
import types

import numpy as np

import concourse.bass as bass
import concourse.mybir as mybir
from concourse import bacc
from concourse.tile import TileContext
from concourse.bass_utils import run_bass_kernel_spmd
from concourse.masks import make_identity

# Problem constants (hardcoded per harness contract).
C = 64
V = 16000
T = 64
B = 8
N_CORES = 8
VS = V // N_CORES          # 2000 vocab rows per core
TB = T * B                 # 512 batched rows
K = C * C                  # 4096 contraction
NK = K // 128              # 32 k-tiles
NM = TB // 128             # 4 tb-tiles
VT = 500                   # vocab tile (psum bank limit: 500 f32 <= 2KB)
NVH = 2                    # vocab halves in main loop
VH = VS // NVH             # 1000

F32 = mybir.dt.float32
F32R = mybir.dt.float32r
BF16 = mybir.dt.bfloat16

W_MODE = os.environ.get("CTRNN_W_MODE", "bf16")  # f32 | f32r | bf16


def _np_wdt(w_mode):
    if w_mode == "bf16":
        import ml_dtypes

        return np.dtype(ml_dtypes.bfloat16)
    return np.dtype(np.float32)


def _bir_wdt(w_mode):
    if w_mode == "bf16":
        return BF16
    if w_mode == "f32r":
        return F32R
    return F32


def build_nc(w_mode=W_MODE, beta_mult=1.0, beta_power=1.0, n_cores=N_CORES):
    """Build the (single-program, SPMD) bass kernel."""
    w_dt = _bir_wdt(w_mode)

    def mm_cast(ap):
        return ap

    nc = bacc.Bacc("TRN2", target_bir_lowering=False, debug=False,
                   num_devices=n_cores)

    ce_d = nc.dram_tensor("ce", [TB, C], F32, kind="ExternalInput")
    aer_d = nc.dram_tensor("aer", [K, TB], w_dt, kind="ExternalInput")
    wt_d = nc.dram_tensor("wt", [K, VS], w_dt, kind="ExternalInput")
    out_d = nc.dram_tensor("out", [TB, VS], F32, kind="ExternalOutput")

    with TileContext(nc) as tc:
        from contextlib import ExitStack

        with (
            tc.tile_pool(name="const", bufs=1) as const_pool,
            tc.tile_pool(name="scan", bufs=1) as scan_pool,
            tc.tile_pool(name="mstage", bufs=1) as mstage_pool,
            tc.tile_pool(name="dram", bufs=1, space="DRAM") as dram_pool,
            tc.tile_pool(name="io", bufs=3) as io_pool,
            tc.tile_pool(name="wst", bufs=3) as w_pool,
            tc.tile_pool(name="ost", bufs=3) as out_pool,
        ):
            ps_tr_ctx = ExitStack()
            ps_tr_pool = ps_tr_ctx.enter_context(
                tc.tile_pool(name="ps_tr", bufs=2, space="PSUM"))
            # ---- constants ----
            identity = const_pool.tile([128, 128], F32)
            make_identity(nc, identity)

            # ---- phase A: load ce in [b, t, c] layout; precompute ne ----
            ce_b = scan_pool.tile([B, T, C], F32)
            nc.sync.dma_start(out=ce_b, in_=ce_d.ap().rearrange(
                "(t b) c -> b t c", b=B))

            sq_all = scan_pool.tile([B, T * C], F32)
            nc.vector.tensor_mul(sq_all, ce_b.rearrange("b t c -> b (t c)"),
                                 ce_b.rearrange("b t c -> b (t c)"))
            ne2 = scan_pool.tile([B, T, 1], F32)
            nc.vector.tensor_reduce(out=ne2, in_=sq_all.rearrange(
                "b (t c) -> b t c", t=T), axis=mybir.AxisListType.X,
                op=mybir.AluOpType.add)
            ne = scan_pool.tile([B, T], F32)
            nc.scalar.activation(out=ne, in_=ne2.rearrange("b t one -> b (t one)"),
                                 func=mybir.ActivationFunctionType.Sqrt)

            # ---- phase B: sequential scan over T ----
            # ctx_seq[:, 0, :] is the zero initial state; step t writes t+1.
            ctx_seq = scan_pool.tile([B, T + 1, C], F32)
            nc.vector.memset(ctx_seq[:, 0, :], 0.0)
            ncur = scan_pool.tile([B, 1], F32)   # ||ctx_t||
            nc.vector.memset(ncur, 0.0)
            ssum = scan_pool.tile([B, 1], F32)
            rinv = scan_pool.tile([B, 1], F32)
            beta = scan_pool.tile([B, 1], F32)
            nc2 = scan_pool.tile([B, 1], F32)
            dvec = scan_pool.tile([B, C], F32)
            sqv = scan_pool.tile([B, C], F32)

            general_beta = (beta_mult != 1.0) or (beta_power != 1.0)
            for t in range(T):
                ne_t = ne[:, t:t + 1]
                ctx_prev = ctx_seq[:, t, :]
                ctx_t = ctx_seq[:, t + 1, :]
                # beta = beta_mult * ne_t / (ne_t + ||ctx_prev||)
                nc.vector.tensor_add(ssum, ne_t, ncur)
                nc.vector.reciprocal(rinv, ssum)
                if beta_mult == 1.0:
                    nc.vector.tensor_mul(beta, ne_t, rinv)
                else:
                    nc.vector.scalar_tensor_tensor(
                        out=beta, in0=ne_t, scalar=float(beta_mult),
                        in1=rinv, op0=mybir.AluOpType.mult,
                        op1=mybir.AluOpType.mult)
                if beta_power != 1.0:
                    nc.scalar.activation(out=beta, in_=beta,
                                         func=mybir.ActivationFunctionType.Ln)
                    nc.vector.tensor_scalar_mul(beta, beta, float(beta_power))
                    nc.scalar.activation(out=beta, in_=beta,
                                         func=mybir.ActivationFunctionType.Exp)
                if general_beta:
                    nc.vector.tensor_scalar_min(beta, beta, 1.0)
                    nc.vector.tensor_scalar_max(beta, beta, 0.0)
                # ctx_t = ctx_prev + beta * (ce_t - ctx_prev)
                nc.vector.tensor_sub(dvec, ce_b[:, t, :], ctx_prev)
                nc.vector.scalar_tensor_tensor(
                    out=ctx_t, in0=dvec, scalar=beta, in1=ctx_prev,
                    op0=mybir.AluOpType.mult, op1=mybir.AluOpType.add)
                # ||ctx_t||
                nc.vector.scalar_tensor_tensor(
                    out=sqv, in0=ctx_t, scalar=1.0, in1=ctx_t,
                    op0=mybir.AluOpType.mult, op1=mybir.AluOpType.mult,
                    accum_out=nc2)
                nc.scalar.activation(out=ncur, in_=nc2,
                                     func=mybir.ActivationFunctionType.Sqrt)

            # ---- phase C: reshape ctx to (t b) rows via DRAM round-trip ----
            ctx_dram = dram_pool.tile([TB, C], F32)
            nc.sync.dma_start(out=ctx_dram.rearrange("(t b) c -> b t c", b=B),
                              in_=ctx_seq[:, 1:, :])

            # ---- phase D: ctxT2[(j,d), tb] = ctx[tb, d]  (j = 0,1) ----
            ctxT2 = mstage_pool.tile([128, TB], F32)
            for m0 in range(NM):
                ctx_tb = io_pool.tile([128, C], F32, name=f"ctx_tb_{m0}",
                                      tag="ctx_tb")
                nc.sync.dma_start(out=ctx_tb,
                                  in_=ctx_dram[m0 * 128:(m0 + 1) * 128, :])
                ctx_ps = ps_tr_pool.tile([C, 128], F32, name=f"ctx_ps_{m0}",
                                         tag="ctx_ps")
                nc.tensor.transpose(ctx_ps, ctx_tb, identity)
                nc.vector.tensor_copy(out=ctxT2[0:C, m0 * 128:(m0 + 1) * 128],
                                      in_=ctx_ps)
            # replicate ctx^T into the lower 64 partitions (cross-partition
            # moves need DMA, not DVE)
            nc.sync.dma_start(out=ctxT2[C:128, :], in_=ctxT2[0:C, :])

            # ---- phase E: mT[(c,d), tb] = aer * ctxT2 ----
            mT = mstage_pool.tile([128, NK, TB], w_dt)
            for k in range(NK):
                aer_k = io_pool.tile([128, TB], w_dt, name=f"aer_{k}",
                                     tag="aer")
                nc.sync.dma_start(out=aer_k,
                                  in_=aer_d.ap()[k * 128:(k + 1) * 128, :])
                nc.vector.tensor_mul(mT[:, k, :], aer_k, ctxT2)

            # ---- phase F: main matmul, W streamed once ----
            ps_tr_ctx.close()  # release transpose psum banks for accumulators
            ps_acc_ctx = ExitStack()
            ps_acc_pool = ps_acc_ctx.enter_context(
                tc.tile_pool(name="ps_acc", bufs=1, space="PSUM"))
            for vh in range(NVH):
                accs = {}
                for m0 in range(NM):
                    for v2 in range(VH // VT):
                        accs[(m0, v2)] = ps_acc_pool.tile(
                            [128, VT], F32, name=f"acc_{m0}_{v2}",
                            tag=f"acc_{m0}_{v2}")
                for k in range(NK):
                    w_k = w_pool.tile([128, VH], w_dt, name=f"w_{vh}_{k}",
                                      tag="w")
                    nc.sync.dma_start(
                        out=w_k,
                        in_=wt_d.ap()[k * 128:(k + 1) * 128,
                                      vh * VH:(vh + 1) * VH])
                    for m0 in range(NM):
                        lhsT = mm_cast(mT[:, k, m0 * 128:(m0 + 1) * 128])
                        for v2 in range(VH // VT):
                            nc.tensor.matmul(
                                accs[(m0, v2)],
                                lhsT,
                                mm_cast(w_k[:, v2 * VT:(v2 + 1) * VT]),
                                start=(k == 0), stop=(k == NK - 1))
                for m0 in range(NM):
                    for v2 in range(VH // VT):
                        o_sb = out_pool.tile([128, VT], F32,
                                             name=f"o_{vh}_{m0}_{v2}",
                                             tag="o")
                        nc.vector.tensor_copy(out=o_sb, in_=accs[(m0, v2)])
                        col = vh * VH + v2 * VT
                        nc.sync.dma_start(
                            out=out_d.ap()[m0 * 128:(m0 + 1) * 128,
                                           col:col + VT],
                            in_=o_sb)
            ps_acc_ctx.close()

    nc.finalize()
    return nc


def build_nc_v2(w_mode="bf16", beta_mult=1.0, beta_power=1.0,
                n_cores=N_CORES):
    """Pipelined builder: scan interleaved per 16-step chunk with ctx
    transpose + mT build + main matmuls (m0-outer, W half resident)."""
    from contextlib import ExitStack

    w_dt = _bir_wdt(w_mode)
    CH = T // NM               # 16 timesteps per chunk == one m0 tile

    nc = bacc.Bacc("TRN2", target_bir_lowering=False, debug=False,
                   num_devices=n_cores)

    ce_d = nc.dram_tensor("ce", [TB, C], F32, kind="ExternalInput")
    aer_d = nc.dram_tensor("aer", [K, TB], w_dt, kind="ExternalInput")
    wt_d = nc.dram_tensor("wt", [K, VS], w_dt, kind="ExternalInput")
    out_d = nc.dram_tensor("out", [TB, VS], F32, kind="ExternalOutput")

    with TileContext(nc) as tc:
        with (
            tc.tile_pool(name="const", bufs=1) as const_pool,
            tc.tile_pool(name="scan", bufs=1) as scan_pool,
            tc.tile_pool(name="mstage", bufs=1) as mstage_pool,
            tc.tile_pool(name="dram", bufs=1, space="DRAM") as dram_pool,
            tc.tile_pool(name="io", bufs=4) as io_pool,
            tc.tile_pool(name="wres", bufs=1) as w_pool,
            tc.tile_pool(name="ost", bufs=3) as out_pool,
            tc.tile_pool(name="ps_tr", bufs=2, space="PSUM") as ps_tr_pool,
            tc.tile_pool(name="ps_acc", bufs=1, space="PSUM") as ps_acc_pool,
        ):
            identity = const_pool.tile([128, 128], F32)
            make_identity(nc, identity)

            # W half 0: issue DMAs first so they stream during the scan.
            w_tiles = {}
            for k in range(NK):
                w_tiles[k] = w_pool.tile([128, VH], w_dt, name=f"w_0_{k}",
                                         tag=f"w{k}")
                nc.sync.dma_start(
                    out=w_tiles[k],
                    in_=wt_d.ap()[k * 128:(k + 1) * 128, 0:VH])

            # scan state/scratch
            ce_b = scan_pool.tile([B, T, C], F32)
            ne = scan_pool.tile([B, T], F32)
            sq_ch = scan_pool.tile([B, CH * C], F32)
            ne2 = scan_pool.tile([B, CH, 1], F32)
            ctx_seq = scan_pool.tile([B, T + 1, C], F32)
            nc.vector.memset(ctx_seq[:, 0, :], 0.0)
            ncur = scan_pool.tile([B, 1], F32)
            nc.vector.memset(ncur, 0.0)
            ssum = scan_pool.tile([B, 1], F32)
            rinv = scan_pool.tile([B, 1], F32)
            beta = scan_pool.tile([B, 1], F32)
            nc2 = scan_pool.tile([B, 1], F32)
            dvec = scan_pool.tile([B, C], F32)
            sqv = scan_pool.tile([B, C], F32)

            ctx_dram = dram_pool.tile([TB, C], F32)
            ctxT2 = mstage_pool.tile([128, TB], F32)
            mT = mstage_pool.tile([128, NK, TB], w_dt)

            general_beta = (beta_mult != 1.0) or (beta_power != 1.0)

            for c_i in range(NM):
                t0 = c_i * CH
                # chunk inputs: ce slice + ne slice (off the critical chain)
                nc.scalar.dma_start(
                    out=ce_b[:, t0:t0 + CH, :],
                    in_=ce_d.ap().rearrange("(t b) c -> b t c", b=B)[
                        :, t0:t0 + CH, :])
                nc.vector.tensor_mul(
                    sq_ch.rearrange("b (t c) -> b t c", t=CH),
                    ce_b[:, t0:t0 + CH, :], ce_b[:, t0:t0 + CH, :])
                nc.vector.tensor_reduce(
                    out=ne2, in_=sq_ch.rearrange("b (t c) -> b t c", t=CH),
                    axis=mybir.AxisListType.X, op=mybir.AluOpType.add)
                nc.scalar.activation(
                    out=ne[:, t0:t0 + CH],
                    in_=ne2.rearrange("b t one -> b (t one)"),
                    func=mybir.ActivationFunctionType.Sqrt)

                for t in range(t0, t0 + CH):
                    ne_t = ne[:, t:t + 1]
                    ctx_prev = ctx_seq[:, t, :]
                    ctx_t = ctx_seq[:, t + 1, :]
                    nc.vector.tensor_add(ssum, ne_t, ncur)
                    nc.vector.reciprocal(rinv, ssum)
                    if beta_mult == 1.0:
                        nc.vector.tensor_mul(beta, ne_t, rinv)
                    else:
                        nc.vector.scalar_tensor_tensor(
                            out=beta, in0=ne_t, scalar=float(beta_mult),
                            in1=rinv, op0=mybir.AluOpType.mult,
                            op1=mybir.AluOpType.mult)
                    if beta_power != 1.0:
                        nc.scalar.activation(
                            out=beta, in_=beta,
                            func=mybir.ActivationFunctionType.Ln)
                        nc.vector.tensor_scalar_mul(beta, beta,
                                                    float(beta_power))
                        nc.scalar.activation(
                            out=beta, in_=beta,
                            func=mybir.ActivationFunctionType.Exp)
                    if general_beta:
                        nc.vector.tensor_scalar_min(beta, beta, 1.0)
                        nc.vector.tensor_scalar_max(beta, beta, 0.0)
                    nc.vector.tensor_sub(dvec, ce_b[:, t, :], ctx_prev)
                    nc.vector.scalar_tensor_tensor(
                        out=ctx_t, in0=dvec, scalar=beta, in1=ctx_prev,
                        op0=mybir.AluOpType.mult, op1=mybir.AluOpType.add)
                    nc.vector.scalar_tensor_tensor(
                        out=sqv, in0=ctx_t, scalar=1.0, in1=ctx_t,
                        op0=mybir.AluOpType.mult, op1=mybir.AluOpType.mult,
                        accum_out=nc2)
                    nc.scalar.activation(
                        out=ncur, in_=nc2,
                        func=mybir.ActivationFunctionType.Sqrt)

                # chunk -> (t b) rows -> ctxT (PE transpose) -> ctxT2 cols
                m0 = c_i
                nc.scalar.dma_start(
                    out=ctx_dram.rearrange("(t b) c -> b t c", b=B)[
                        :, t0:t0 + CH, :],
                    in_=ctx_seq[:, t0 + 1:t0 + CH + 1, :])
                ctx_tb = io_pool.tile([128, C], F32, name=f"ctx_tb_{m0}",
                                      tag="ctx_tb")
                nc.scalar.dma_start(out=ctx_tb,
                                    in_=ctx_dram[m0 * 128:(m0 + 1) * 128, :])
                ctx_ps = ps_tr_pool.tile([C, 128], F32, name=f"ctx_ps_{m0}",
                                         tag="ctx_ps")
                nc.tensor.transpose(ctx_ps, ctx_tb, identity)
                nc.vector.tensor_copy(
                    out=ctxT2[0:C, m0 * 128:(m0 + 1) * 128], in_=ctx_ps)
                nc.scalar.dma_start(
                    out=ctxT2[C:128, m0 * 128:(m0 + 1) * 128],
                    in_=ctxT2[0:C, m0 * 128:(m0 + 1) * 128])

                # mT column block for this m0
                for k in range(NK):
                    aer_km = io_pool.tile([128, 128], w_dt,
                                          name=f"aer_{m0}_{k}", tag="aer")
                    nc.scalar.dma_start(
                        out=aer_km,
                        in_=aer_d.ap()[k * 128:(k + 1) * 128,
                                       m0 * 128:(m0 + 1) * 128])
                    nc.vector.tensor_mul(
                        mT[:, k, m0 * 128:(m0 + 1) * 128], aer_km,
                        ctxT2[:, m0 * 128:(m0 + 1) * 128])

            # main matmuls: vh passes, m0-outer, k inner (W resident)
            for vh in range(NVH):
                if vh > 0:
                    for k in range(NK):
                        w_tiles[k] = w_pool.tile([128, VH], w_dt,
                                                 name=f"w_{vh}_{k}",
                                                 tag=f"w{k}")
                        nc.sync.dma_start(
                            out=w_tiles[k],
                            in_=wt_d.ap()[k * 128:(k + 1) * 128,
                                          vh * VH:(vh + 1) * VH])
                for m0 in range(NM):
                    accs = []
                    for v2 in range(VH // VT):
                        accs.append(ps_acc_pool.tile(
                            [128, VT], F32, name=f"acc_{vh}_{m0}_{v2}",
                            tag=f"acc_{m0 % 2}_{v2}"))
                    for k in range(NK):
                        lhsT = mT[:, k, m0 * 128:(m0 + 1) * 128]
                        for v2 in range(VH // VT):
                            nc.tensor.matmul(
                                accs[v2], lhsT,
                                w_tiles[k][:, v2 * VT:(v2 + 1) * VT],
                                start=(k == 0), stop=(k == NK - 1))
                    for v2 in range(VH // VT):
                        o_sb = out_pool.tile([128, VT], F32,
                                             name=f"o_{vh}_{m0}_{v2}",
                                             tag="o")
                        nc.vector.tensor_copy(out=o_sb, in_=accs[v2])
                        col = vh * VH + v2 * VT
                        nc.sync.dma_start(
                            out=out_d.ap()[m0 * 128:(m0 + 1) * 128,
                                           col:col + VT],
                            in_=o_sb)

    nc.finalize()
    return nc


def host_prep(tokens, emb_ctx, emb_act, W_a, w_mode=W_MODE):
    """Gather embeddings, build the replicated ae^T and the W shards."""
    wnp = _np_wdt(w_mode)
    tok = np.asarray(tokens).astype(np.int64).reshape(-1)  # [T*B], t-major
    ce = np.ascontiguousarray(np.asarray(emb_ctx, dtype=np.float32)[tok])
    ae = np.asarray(emb_act, dtype=np.float32)[tok]        # [TB, C]
    # aer[(c,d), tb] = ae[tb, c]
    aer = np.ascontiguousarray(
        np.repeat(ae.T, C, axis=0)).astype(wnp)            # [C*C, TB]
    W3 = np.asarray(W_a, dtype=np.float32).reshape(V, C, C)
    in_maps = []
    for s in range(N_CORES):
        ws = np.ascontiguousarray(
            W3[s * VS:(s + 1) * VS].transpose(1, 2, 0).reshape(K, VS)
        ).astype(wnp)                                      # [(c,d), v_local]
        in_maps.append({"ce": ce, "aer": aer, "wt": ws})
    return in_maps


_NC_CACHE = {}


def _get_nc(w_mode, beta_mult, beta_power):
    key = (w_mode, float(beta_mult), float(beta_power))
    if key not in _NC_CACHE:
        builder = build_nc_v2 if w_mode == "bf16" else build_nc
        _NC_CACHE[key] = builder(w_mode, *key[1:])
    return _NC_CACHE[key]


def install_ntff_shim():
    """Optional: register the axon NTFF profiling hook (for tracing)."""
    if "antenv.axon_hooks" in sys.modules:
        return
    m = types.ModuleType("antenv.axon_hooks")
    state = {"hook": None}
    m.get_axon_ntff_profile_hook = lambda: state["hook"]
    m.set_axon_ntff_profile_hook = lambda h: state.update(hook=h)
    sys.modules["antenv.axon_hooks"] = m
    try:
        from trn_agent_boot.trn_boot import _ntff_profile_via_ctypes

        state["hook"] = _ntff_profile_via_ctypes("/opt/axon/libaxon_pjrt.so")
    except Exception:
        pass


def run_hw(tokens, emb_ctx, emb_act, W_a, beta_mult, beta_power,
           w_mode=W_MODE, trace=False):
    if trace:
        install_ntff_shim()
    nc = _get_nc(w_mode, float(beta_mult), float(beta_power))
    in_maps = host_prep(tokens, emb_ctx, emb_act, W_a, w_mode)
    res = run_bass_kernel_spmd(nc, in_maps, core_ids=list(range(N_CORES)),
                               trace=trace)
    out = np.concatenate([res.results[s]["out"] for s in range(N_CORES)],
                         axis=1)                           # [TB, V]
    return out.reshape(T, B, V), res


def kernel(tokens, emb_ctx, emb_act, W_a, beta_mult, beta_power):
    out, _ = run_hw(tokens, emb_ctx, emb_act, W_a,
                    float(np.asarray(beta_mult)), float(np.asarray(beta_power)))
    return out


# revision 12
# speedup vs baseline: 1.4324x; 1.1232x over previous
"""Trainium2 Bass kernel for nn_ContextCTRNN.

Math: per timestep t, ctx is blended with the token's context embedding via a
norm-gated beta, then out[b,v] = ae_b^T @ W3[v] @ ctx_b.  The bilinear readout
is restructured as one big matmul:

    out[tb, v] = sum_{c,d} W3[v,c,d] * ae[tb,c] * ctx[tb,d]
               = (m @ W2^T)[tb, v],   m[tb, (c,d)] = ae[tb,c]*ctx[tb,d]

with tb = t*B+b (the scan only affects the tiny [B,C] ctx state, so all T*B
rows are batched).  Sharding: vocab (V) split across 8 cores; each core gets
W2^T's shard [C*C, V/8] plus replicated activations, computes out[:, vshard],
and the host concatenates.

Device kernel per core:
  1. sequential ctx scan over T on [B, C] tiles (DVE/ACT)
  2. PE-transpose ctx -> ctxT2 [128, TB] (two stacked copies of ctx^T)
  3. mT[(c,d), tb] = aeT_rep (host-prepped, DMA'd) * ctxT2   (DVE)
  4. out_psum[tb, v] += mT_k^T @ Wt_k  accumulated over 32 k-tiles (PE)
"""

import os
import sys
import types

import numpy as np

import concourse.bass as bass
import concourse.mybir as mybir
from concourse import bacc
from concourse.tile import TileContext
from concourse.bass_utils import run_bass_kernel_spmd
from concourse.masks import make_identity

# Problem constants (hardcoded per harness contract).
C = 64
V = 16000
T = 64
B = 8
N_CORES = 8
VS = V // N_CORES          # 2000 vocab rows per core
TB = T * B                 # 512 batched rows
K = C * C                  # 4096 contraction
NK = K // 128              # 32 k-tiles
NM = TB // 128             # 4 tb-tiles
VT = 500                   # vocab tile (psum bank limit: 500 f32 <= 2KB)
NVH = 2                    # vocab halves in main loop
VH = VS // NVH             # 1000

F32 = mybir.dt.float32
F32R = mybir.dt.float32r
BF16 = mybir.dt.bfloat16

W_MODE = os.environ.get("CTRNN_W_MODE", "bf16")  # f32 | f32r | bf16


def _np_wdt(w_mode):
    if w_mode == "bf16":
        import ml_dtypes

        return np.dtype(ml_dtypes.bfloat16)
    return np.dtype(np.float32)


def _bir_wdt(w_mode):
    if w_mode == "bf16":
        return BF16
    if w_mode == "f32r":
        return F32R
    return F32


def build_nc(w_mode=W_MODE, beta_mult=1.0, beta_power=1.0, n_cores=N_CORES):
    """Build the (single-program, SPMD) bass kernel."""
    w_dt = _bir_wdt(w_mode)

    def mm_cast(ap):
        return ap

    nc = bacc.Bacc("TRN2", target_bir_lowering=False, debug=False,
                   num_devices=n_cores)

    ce_d = nc.dram_tensor("ce", [TB, C], F32, kind="ExternalInput")
    aer_d = nc.dram_tensor("aer", [K, TB], w_dt, kind="ExternalInput")
    wt_d = nc.dram_tensor("wt", [K, VS], w_dt, kind="ExternalInput")
    out_d = nc.dram_tensor("out", [TB, VS], F32, kind="ExternalOutput")

    with TileContext(nc) as tc:
        from contextlib import ExitStack

        with (
            tc.tile_pool(name="const", bufs=1) as const_pool,
            tc.tile_pool(name="scan", bufs=1) as scan_pool,
            tc.tile_pool(name="mstage", bufs=1) as mstage_pool,
            tc.tile_pool(name="dram", bufs=1, space="DRAM") as dram_pool,
            tc.tile_pool(name="io", bufs=3) as io_pool,
            tc.tile_pool(name="wst", bufs=3) as w_pool,
            tc.tile_pool(name="ost", bufs=3) as out_pool,
        ):
            ps_tr_ctx = ExitStack()
            ps_tr_pool = ps_tr_ctx.enter_context(
                tc.tile_pool(name="ps_tr", bufs=2, space="PSUM"))
            # ---- constants ----
            identity = const_pool.tile([128, 128], F32)
            make_identity(nc, identity)

            # ---- phase A: load ce in [b, t, c] layout; precompute ne ----
            ce_b = scan_pool.tile([B, T, C], F32)
            nc.sync.dma_start(out=ce_b, in_=ce_d.ap().rearrange(
                "(t b) c -> b t c", b=B))

            sq_all = scan_pool.tile([B, T * C], F32)
            nc.vector.tensor_mul(sq_all, ce_b.rearrange("b t c -> b (t c)"),
                                 ce_b.rearrange("b t c -> b (t c)"))
            ne2 = scan_pool.tile([B, T, 1], F32)
            nc.vector.tensor_reduce(out=ne2, in_=sq_all.rearrange(
                "b (t c) -> b t c", t=T), axis=mybir.AxisListType.X,
                op=mybir.AluOpType.add)
            ne = scan_pool.tile([B, T], F32)
            nc.scalar.activation(out=ne, in_=ne2.rearrange("b t one -> b (t one)"),
                                 func=mybir.ActivationFunctionType.Sqrt)

            # ---- phase B: sequential scan over T ----
            # ctx_seq[:, 0, :] is the zero initial state; step t writes t+1.
            ctx_seq = scan_pool.tile([B, T + 1, C], F32)
            nc.vector.memset(ctx_seq[:, 0, :], 0.0)
            ncur = scan_pool.tile([B, 1], F32)   # ||ctx_t||
            nc.vector.memset(ncur, 0.0)
            ssum = scan_pool.tile([B, 1], F32)
            rinv = scan_pool.tile([B, 1], F32)
            beta = scan_pool.tile([B, 1], F32)
            nc2 = scan_pool.tile([B, 1], F32)
            dvec = scan_pool.tile([B, C], F32)
            sqv = scan_pool.tile([B, C], F32)

            general_beta = (beta_mult != 1.0) or (beta_power != 1.0)
            for t in range(T):
                ne_t = ne[:, t:t + 1]
                ctx_prev = ctx_seq[:, t, :]
                ctx_t = ctx_seq[:, t + 1, :]
                # beta = beta_mult * ne_t / (ne_t + ||ctx_prev||)
                nc.vector.tensor_add(ssum, ne_t, ncur)
                nc.vector.reciprocal(rinv, ssum)
                if beta_mult == 1.0:
                    nc.vector.tensor_mul(beta, ne_t, rinv)
                else:
                    nc.vector.scalar_tensor_tensor(
                        out=beta, in0=ne_t, scalar=float(beta_mult),
                        in1=rinv, op0=mybir.AluOpType.mult,
                        op1=mybir.AluOpType.mult)
                if beta_power != 1.0:
                    nc.scalar.activation(out=beta, in_=beta,
                                         func=mybir.ActivationFunctionType.Ln)
                    nc.vector.tensor_scalar_mul(beta, beta, float(beta_power))
                    nc.scalar.activation(out=beta, in_=beta,
                                         func=mybir.ActivationFunctionType.Exp)
                if general_beta:
                    nc.vector.tensor_scalar_min(beta, beta, 1.0)
                    nc.vector.tensor_scalar_max(beta, beta, 0.0)
                # ctx_t = ctx_prev + beta * (ce_t - ctx_prev)
                nc.vector.tensor_sub(dvec, ce_b[:, t, :], ctx_prev)
                nc.vector.scalar_tensor_tensor(
                    out=ctx_t, in0=dvec, scalar=beta, in1=ctx_prev,
                    op0=mybir.AluOpType.mult, op1=mybir.AluOpType.add)
                # ||ctx_t||
                nc.vector.scalar_tensor_tensor(
                    out=sqv, in0=ctx_t, scalar=1.0, in1=ctx_t,
                    op0=mybir.AluOpType.mult, op1=mybir.AluOpType.mult,
                    accum_out=nc2)
                nc.scalar.activation(out=ncur, in_=nc2,
                                     func=mybir.ActivationFunctionType.Sqrt)

            # ---- phase C: reshape ctx to (t b) rows via DRAM round-trip ----
            ctx_dram = dram_pool.tile([TB, C], F32)
            nc.sync.dma_start(out=ctx_dram.rearrange("(t b) c -> b t c", b=B),
                              in_=ctx_seq[:, 1:, :])

            # ---- phase D: ctxT2[(j,d), tb] = ctx[tb, d]  (j = 0,1) ----
            ctxT2 = mstage_pool.tile([128, TB], F32)
            for m0 in range(NM):
                ctx_tb = io_pool.tile([128, C], F32, name=f"ctx_tb_{m0}",
                                      tag="ctx_tb")
                nc.sync.dma_start(out=ctx_tb,
                                  in_=ctx_dram[m0 * 128:(m0 + 1) * 128, :])
                ctx_ps = ps_tr_pool.tile([C, 128], F32, name=f"ctx_ps_{m0}",
                                         tag="ctx_ps")
                nc.tensor.transpose(ctx_ps, ctx_tb, identity)
                nc.vector.tensor_copy(out=ctxT2[0:C, m0 * 128:(m0 + 1) * 128],
                                      in_=ctx_ps)
            # replicate ctx^T into the lower 64 partitions (cross-partition
            # moves need DMA, not DVE)
            nc.sync.dma_start(out=ctxT2[C:128, :], in_=ctxT2[0:C, :])

            # ---- phase E: mT[(c,d), tb] = aer * ctxT2 ----
            mT = mstage_pool.tile([128, NK, TB], w_dt)
            for k in range(NK):
                aer_k = io_pool.tile([128, TB], w_dt, name=f"aer_{k}",
                                     tag="aer")
                nc.sync.dma_start(out=aer_k,
                                  in_=aer_d.ap()[k * 128:(k + 1) * 128, :])
                nc.vector.tensor_mul(mT[:, k, :], aer_k, ctxT2)

            # ---- phase F: main matmul, W streamed once ----
            ps_tr_ctx.close()  # release transpose psum banks for accumulators
            ps_acc_ctx = ExitStack()
            ps_acc_pool = ps_acc_ctx.enter_context(
                tc.tile_pool(name="ps_acc", bufs=1, space="PSUM"))
            for vh in range(NVH):
                accs = {}
                for m0 in range(NM):
                    for v2 in range(VH // VT):
                        accs[(m0, v2)] = ps_acc_pool.tile(
                            [128, VT], F32, name=f"acc_{m0}_{v2}",
                            tag=f"acc_{m0}_{v2}")
                for k in range(NK):
                    w_k = w_pool.tile([128, VH], w_dt, name=f"w_{vh}_{k}",
                                      tag="w")
                    nc.sync.dma_start(
                        out=w_k,
                        in_=wt_d.ap()[k * 128:(k + 1) * 128,
                                      vh * VH:(vh + 1) * VH])
                    for m0 in range(NM):
                        lhsT = mm_cast(mT[:, k, m0 * 128:(m0 + 1) * 128])
                        for v2 in range(VH // VT):
                            nc.tensor.matmul(
                                accs[(m0, v2)],
                                lhsT,
                                mm_cast(w_k[:, v2 * VT:(v2 + 1) * VT]),
                                start=(k == 0), stop=(k == NK - 1))
                for m0 in range(NM):
                    for v2 in range(VH // VT):
                        o_sb = out_pool.tile([128, VT], F32,
                                             name=f"o_{vh}_{m0}_{v2}",
                                             tag="o")
                        nc.vector.tensor_copy(out=o_sb, in_=accs[(m0, v2)])
                        col = vh * VH + v2 * VT
                        nc.sync.dma_start(
                            out=out_d.ap()[m0 * 128:(m0 + 1) * 128,
                                           col:col + VT],
                            in_=o_sb)
            ps_acc_ctx.close()

    nc.finalize()
    return nc


def build_nc_v2(w_mode="bf16", beta_mult=1.0, beta_power=1.0,
                n_cores=N_CORES):
    """Pipelined builder: scan interleaved per 16-step chunk with ctx
    transpose + mT build + main matmuls (m0-outer, W half resident)."""
    from contextlib import ExitStack

    w_dt = _bir_wdt(w_mode)
    CH = T // NM               # 16 timesteps per chunk == one m0 tile

    nc = bacc.Bacc("TRN2", target_bir_lowering=False, debug=False,
                   num_devices=n_cores)

    ce_d = nc.dram_tensor("ce", [TB, C], F32, kind="ExternalInput")
    aer_d = nc.dram_tensor("aer", [K, TB], w_dt, kind="ExternalInput")
    wt_d = nc.dram_tensor("wt", [K, VS], w_dt, kind="ExternalInput")
    out_d = nc.dram_tensor("out", [TB, VS], F32, kind="ExternalOutput")

    with TileContext(nc) as tc:
        with (
            tc.tile_pool(name="const", bufs=1) as const_pool,
            tc.tile_pool(name="scan", bufs=1) as scan_pool,
            tc.tile_pool(name="mstage", bufs=1) as mstage_pool,
            tc.tile_pool(name="dram", bufs=1, space="DRAM") as dram_pool,
            tc.tile_pool(name="io", bufs=4) as io_pool,
            tc.tile_pool(name="wres", bufs=1) as w_pool,
            tc.tile_pool(name="ost", bufs=3) as out_pool,
            tc.tile_pool(name="ps_tr", bufs=2, space="PSUM") as ps_tr_pool,
            tc.tile_pool(name="ps_acc", bufs=1, space="PSUM") as ps_acc_pool,
        ):
            identity = const_pool.tile([128, 128], F32)
            make_identity(nc, identity)

            # W half 0: issue DMAs first so they stream during the scan.
            w_tiles = {}
            for k in range(NK):
                w_tiles[k] = w_pool.tile([128, VH], w_dt, name=f"w_0_{k}",
                                         tag=f"w{k}")
                nc.sync.dma_start(
                    out=w_tiles[k],
                    in_=wt_d.ap()[k * 128:(k + 1) * 128, 0:VH])

            # whole aer in one bulk DMA (no per-chunk descriptor traffic)
            aer_sb = mstage_pool.tile([128, NK, TB], w_dt)
            nc.sync.dma_start(
                out=aer_sb,
                in_=aer_d.ap().rearrange("(k p) tb -> p k tb", p=128))

            # scan state/scratch
            ce_b = scan_pool.tile([B, T, C], F32)
            ne = scan_pool.tile([B, T], F32)
            sq_ch = scan_pool.tile([B, CH * C], F32)
            ne2 = scan_pool.tile([B, CH, 1], F32)
            ctx_seq = scan_pool.tile([B, T + 1, C], F32)
            nc.vector.memset(ctx_seq[:, 0, :], 0.0)
            ncur = scan_pool.tile([B, 1], F32)
            nc.vector.memset(ncur, 0.0)
            ssum = scan_pool.tile([B, 1], F32)
            rinv = scan_pool.tile([B, 1], F32)
            beta = scan_pool.tile([B, 1], F32)
            nc2 = scan_pool.tile([B, 1], F32)
            dvec = scan_pool.tile([B, C], F32)
            sqv = scan_pool.tile([B, C], F32)

            ctx_dram = dram_pool.tile([TB, C], F32)
            ctxT2 = mstage_pool.tile([128, TB], F32)
            mT = mstage_pool.tile([128, NK, TB], w_dt)

            general_beta = (beta_mult != 1.0) or (beta_power != 1.0)

            for c_i in range(NM):
                t0 = c_i * CH
                # chunk inputs: ce slice + ne slice (off the critical chain)
                nc.scalar.dma_start(
                    out=ce_b[:, t0:t0 + CH, :],
                    in_=ce_d.ap().rearrange("(t b) c -> b t c", b=B)[
                        :, t0:t0 + CH, :])
                nc.vector.tensor_mul(
                    sq_ch.rearrange("b (t c) -> b t c", t=CH),
                    ce_b[:, t0:t0 + CH, :], ce_b[:, t0:t0 + CH, :])
                nc.vector.tensor_reduce(
                    out=ne2, in_=sq_ch.rearrange("b (t c) -> b t c", t=CH),
                    axis=mybir.AxisListType.X, op=mybir.AluOpType.add)
                nc.scalar.activation(
                    out=ne[:, t0:t0 + CH],
                    in_=ne2.rearrange("b t one -> b (t one)"),
                    func=mybir.ActivationFunctionType.Sqrt)

                for t in range(t0, t0 + CH):
                    ne_t = ne[:, t:t + 1]
                    ctx_prev = ctx_seq[:, t, :]
                    ctx_t = ctx_seq[:, t + 1, :]
                    nc.vector.tensor_add(ssum, ne_t, ncur)
                    nc.vector.reciprocal(rinv, ssum)
                    if beta_mult == 1.0:
                        nc.vector.tensor_mul(beta, ne_t, rinv)
                    else:
                        nc.vector.scalar_tensor_tensor(
                            out=beta, in0=ne_t, scalar=float(beta_mult),
                            in1=rinv, op0=mybir.AluOpType.mult,
                            op1=mybir.AluOpType.mult)
                    if beta_power != 1.0:
                        nc.scalar.activation(
                            out=beta, in_=beta,
                            func=mybir.ActivationFunctionType.Ln)
                        nc.vector.tensor_scalar_mul(beta, beta,
                                                    float(beta_power))
                        nc.scalar.activation(
                            out=beta, in_=beta,
                            func=mybir.ActivationFunctionType.Exp)
                    if general_beta:
                        nc.vector.tensor_scalar_min(beta, beta, 1.0)
                        nc.vector.tensor_scalar_max(beta, beta, 0.0)
                    nc.vector.tensor_sub(dvec, ce_b[:, t, :], ctx_prev)
                    nc.vector.scalar_tensor_tensor(
                        out=ctx_t, in0=dvec, scalar=beta, in1=ctx_prev,
                        op0=mybir.AluOpType.mult, op1=mybir.AluOpType.add)
                    nc.vector.scalar_tensor_tensor(
                        out=sqv, in0=ctx_t, scalar=1.0, in1=ctx_t,
                        op0=mybir.AluOpType.mult, op1=mybir.AluOpType.mult,
                        accum_out=nc2)
                    nc.scalar.activation(
                        out=ncur, in_=nc2,
                        func=mybir.ActivationFunctionType.Sqrt)

                # chunk -> (t b) rows -> ctxT (PE transpose) -> ctxT2 cols
                m0 = c_i
                nc.scalar.dma_start(
                    out=ctx_dram.rearrange("(t b) c -> b t c", b=B)[
                        :, t0:t0 + CH, :],
                    in_=ctx_seq[:, t0 + 1:t0 + CH + 1, :])
                ctx_tb = io_pool.tile([128, C], F32, name=f"ctx_tb_{m0}",
                                      tag="ctx_tb")
                nc.scalar.dma_start(out=ctx_tb,
                                    in_=ctx_dram[m0 * 128:(m0 + 1) * 128, :])
                ctx_ps = ps_tr_pool.tile([C, 128], F32, name=f"ctx_ps_{m0}",
                                         tag="ctx_ps")
                nc.tensor.transpose(ctx_ps, ctx_tb, identity)
                nc.vector.tensor_copy(
                    out=ctxT2[0:C, m0 * 128:(m0 + 1) * 128], in_=ctx_ps)
                nc.scalar.dma_start(
                    out=ctxT2[C:128, m0 * 128:(m0 + 1) * 128],
                    in_=ctxT2[0:C, m0 * 128:(m0 + 1) * 128])

                # mT column block for this m0
                for k in range(NK):
                    nc.vector.tensor_mul(
                        mT[:, k, m0 * 128:(m0 + 1) * 128],
                        aer_sb[:, k, m0 * 128:(m0 + 1) * 128],
                        ctxT2[:, m0 * 128:(m0 + 1) * 128])

            # main matmuls: vh passes, m0-outer, k inner (W resident)
            for vh in range(NVH):
                if vh > 0:
                    for k in range(NK):
                        w_tiles[k] = w_pool.tile([128, VH], w_dt,
                                                 name=f"w_{vh}_{k}",
                                                 tag=f"w{k}")
                        nc.sync.dma_start(
                            out=w_tiles[k],
                            in_=wt_d.ap()[k * 128:(k + 1) * 128,
                                          vh * VH:(vh + 1) * VH])
                for m0 in range(NM):
                    accs = []
                    for v2 in range(VH // VT):
                        accs.append(ps_acc_pool.tile(
                            [128, VT], F32, name=f"acc_{vh}_{m0}_{v2}",
                            tag=f"acc_{m0 % 2}_{v2}"))
                    for k in range(NK):
                        lhsT = mT[:, k, m0 * 128:(m0 + 1) * 128]
                        for v2 in range(VH // VT):
                            nc.tensor.matmul(
                                accs[v2], lhsT,
                                w_tiles[k][:, v2 * VT:(v2 + 1) * VT],
                                start=(k == 0), stop=(k == NK - 1))
                    for v2 in range(VH // VT):
                        o_sb = out_pool.tile([128, VT], F32,
                                             name=f"o_{vh}_{m0}_{v2}",
                                             tag="o")
                        nc.vector.tensor_copy(out=o_sb, in_=accs[v2])
                        col = vh * VH + v2 * VT
                        nc.sync.dma_start(
                            out=out_d.ap()[m0 * 128:(m0 + 1) * 128,
                                           col:col + VT],
                            in_=o_sb)

    nc.finalize()
    return nc


def host_prep(tokens, emb_ctx, emb_act, W_a, w_mode=W_MODE):
    """Gather embeddings, build the replicated ae^T and the W shards."""
    wnp = _np_wdt(w_mode)
    tok = np.asarray(tokens).astype(np.int64).reshape(-1)  # [T*B], t-major
    ce = np.ascontiguousarray(np.asarray(emb_ctx, dtype=np.float32)[tok])
    ae = np.asarray(emb_act, dtype=np.float32)[tok]        # [TB, C]
    # aer[(c,d), tb] = ae[tb, c]
    aer = np.ascontiguousarray(
        np.repeat(ae.T, C, axis=0)).astype(wnp)            # [C*C, TB]
    W3 = np.asarray(W_a, dtype=np.float32).reshape(V, C, C)
    in_maps = []
    for s in range(N_CORES):
        ws = np.ascontiguousarray(
            W3[s * VS:(s + 1) * VS].transpose(1, 2, 0).reshape(K, VS)
        ).astype(wnp)                                      # [(c,d), v_local]
        in_maps.append({"ce": ce, "aer": aer, "wt": ws})
    return in_maps


_NC_CACHE = {}


def _get_nc(w_mode, beta_mult, beta_power):
    key = (w_mode, float(beta_mult), float(beta_power))
    if key not in _NC_CACHE:
        builder = build_nc_v2 if w_mode == "bf16" else build_nc
        _NC_CACHE[key] = builder(w_mode, *key[1:])
    return _NC_CACHE[key]


def install_ntff_shim():
    """Optional: register the axon NTFF profiling hook (for tracing)."""
    if "antenv.axon_hooks" in sys.modules:
        return
    m = types.ModuleType("antenv.axon_hooks")
    state = {"hook": None}
    m.get_axon_ntff_profile_hook = lambda: state["hook"]
    m.set_axon_ntff_profile_hook = lambda h: state.update(hook=h)
    sys.modules["antenv.axon_hooks"] = m
    try:
        from trn_agent_boot.trn_boot import _ntff_profile_via_ctypes

        state["hook"] = _ntff_profile_via_ctypes("/opt/axon/libaxon_pjrt.so")
    except Exception:
        pass


def run_hw(tokens, emb_ctx, emb_act, W_a, beta_mult, beta_power,
           w_mode=W_MODE, trace=False):
    if trace:
        install_ntff_shim()
    nc = _get_nc(w_mode, float(beta_mult), float(beta_power))
    in_maps = host_prep(tokens, emb_ctx, emb_act, W_a, w_mode)
    res = run_bass_kernel_spmd(nc, in_maps, core_ids=list(range(N_CORES)),
                               trace=trace)
    out = np.concatenate([res.results[s]["out"] for s in range(N_CORES)],
                         axis=1)                           # [TB, V]
    return out.reshape(T, B, V), res


def kernel(tokens, emb_ctx, emb_act, W_a, beta_mult, beta_power):
    out, _ = run_hw(tokens, emb_ctx, emb_act, W_a,
                    float(np.asarray(beta_mult)), float(np.asarray(beta_power)))
    return out


# revision 16
# speedup vs baseline: 1.4681x; 1.0249x over previous
"""Trainium2 Bass kernel for nn_ContextCTRNN.

Math: per timestep t, ctx is blended with the token's context embedding via a
norm-gated beta, then out[b,v] = ae_b^T @ W3[v] @ ctx_b.  The bilinear readout
is restructured as one big matmul:

    out[tb, v] = sum_{c,d} W3[v,c,d] * ae[tb,c] * ctx[tb,d]
               = (m @ W2^T)[tb, v],   m[tb, (c,d)] = ae[tb,c]*ctx[tb,d]

with tb = t*B+b (the scan only affects the tiny [B,C] ctx state, so all T*B
rows are batched).  Sharding: vocab (V) split across 8 cores; each core gets
W2^T's shard [C*C, V/8] plus replicated activations, computes out[:, vshard],
and the host concatenates.

Device kernel per core:
  1. sequential ctx scan over T on [B, C] tiles (DVE/ACT)
  2. PE-transpose ctx -> ctxT2 [128, TB] (two stacked copies of ctx^T)
  3. mT[(c,d), tb] = aeT_rep (host-prepped, DMA'd) * ctxT2   (DVE)
  4. out_psum[tb, v] += mT_k^T @ Wt_k  accumulated over 32 k-tiles (PE)
"""

import os
import sys
import types

import numpy as np

import concourse.bass as bass
import concourse.mybir as mybir
from concourse import bacc
from concourse.tile import TileContext
from concourse.bass_utils import run_bass_kernel_spmd
from concourse.masks import make_identity

# Problem constants (hardcoded per harness contract).
C = 64
V = 16000
T = 64
B = 8
N_CORES = 8
VS = V // N_CORES          # 2000 vocab rows per core
TB = T * B                 # 512 batched rows
K = C * C                  # 4096 contraction
NK = K // 128              # 32 k-tiles
NM = TB // 128             # 4 tb-tiles
VT = 500                   # vocab tile (psum bank limit: 500 f32 <= 2KB)
NVH = 2                    # vocab halves in main loop
VH = VS // NVH             # 1000

F32 = mybir.dt.float32
F32R = mybir.dt.float32r
BF16 = mybir.dt.bfloat16

W_MODE = os.environ.get("CTRNN_W_MODE", "bf16")  # f32 | f32r | bf16
USE_DIVIDE = os.environ.get("CTRNN_DIVIDE", "0") == "1"  # DVE has no divide


def _np_wdt(w_mode):
    if w_mode == "bf16":
        import ml_dtypes

        return np.dtype(ml_dtypes.bfloat16)
    return np.dtype(np.float32)


def _bir_wdt(w_mode):
    if w_mode == "bf16":
        return BF16
    if w_mode == "f32r":
        return F32R
    return F32


def build_nc(w_mode=W_MODE, beta_mult=1.0, beta_power=1.0, n_cores=N_CORES):
    """Build the (single-program, SPMD) bass kernel."""
    w_dt = _bir_wdt(w_mode)

    def mm_cast(ap):
        return ap

    nc = bacc.Bacc("TRN2", target_bir_lowering=False, debug=False,
                   num_devices=n_cores)

    ce_d = nc.dram_tensor("ce", [TB, C], F32, kind="ExternalInput")
    aer_d = nc.dram_tensor("aer", [K, TB], w_dt, kind="ExternalInput")
    wt_d = nc.dram_tensor("wt", [K, VS], w_dt, kind="ExternalInput")
    out_d = nc.dram_tensor("out", [TB, VS], F32, kind="ExternalOutput")

    with TileContext(nc) as tc:
        from contextlib import ExitStack

        with (
            tc.tile_pool(name="const", bufs=1) as const_pool,
            tc.tile_pool(name="scan", bufs=1) as scan_pool,
            tc.tile_pool(name="mstage", bufs=1) as mstage_pool,
            tc.tile_pool(name="dram", bufs=1, space="DRAM") as dram_pool,
            tc.tile_pool(name="io", bufs=3) as io_pool,
            tc.tile_pool(name="wst", bufs=3) as w_pool,
            tc.tile_pool(name="ost", bufs=3) as out_pool,
        ):
            ps_tr_ctx = ExitStack()
            ps_tr_pool = ps_tr_ctx.enter_context(
                tc.tile_pool(name="ps_tr", bufs=2, space="PSUM"))
            # ---- constants ----
            identity = const_pool.tile([128, 128], F32)
            make_identity(nc, identity)

            # ---- phase A: load ce in [b, t, c] layout; precompute ne ----
            ce_b = scan_pool.tile([B, T, C], F32)
            nc.sync.dma_start(out=ce_b, in_=ce_d.ap().rearrange(
                "(t b) c -> b t c", b=B))

            sq_all = scan_pool.tile([B, T * C], F32)
            nc.vector.tensor_mul(sq_all, ce_b.rearrange("b t c -> b (t c)"),
                                 ce_b.rearrange("b t c -> b (t c)"))
            ne2 = scan_pool.tile([B, T, 1], F32)
            nc.vector.tensor_reduce(out=ne2, in_=sq_all.rearrange(
                "b (t c) -> b t c", t=T), axis=mybir.AxisListType.X,
                op=mybir.AluOpType.add)
            ne = scan_pool.tile([B, T], F32)
            nc.scalar.activation(out=ne, in_=ne2.rearrange("b t one -> b (t one)"),
                                 func=mybir.ActivationFunctionType.Sqrt)

            # ---- phase B: sequential scan over T ----
            # ctx_seq[:, 0, :] is the zero initial state; step t writes t+1.
            ctx_seq = scan_pool.tile([B, T + 1, C], F32)
            nc.vector.memset(ctx_seq[:, 0, :], 0.0)
            ncur = scan_pool.tile([B, 1], F32)   # ||ctx_t||
            nc.vector.memset(ncur, 0.0)
            ssum = scan_pool.tile([B, 1], F32)
            rinv = scan_pool.tile([B, 1], F32)
            beta = scan_pool.tile([B, 1], F32)
            nc2 = scan_pool.tile([B, 1], F32)
            dvec = scan_pool.tile([B, C], F32)
            sqv = scan_pool.tile([B, C], F32)

            general_beta = (beta_mult != 1.0) or (beta_power != 1.0)
            for t in range(T):
                ne_t = ne[:, t:t + 1]
                ctx_prev = ctx_seq[:, t, :]
                ctx_t = ctx_seq[:, t + 1, :]
                # beta = beta_mult * ne_t / (ne_t + ||ctx_prev||)
                nc.vector.tensor_add(ssum, ne_t, ncur)
                nc.vector.reciprocal(rinv, ssum)
                if beta_mult == 1.0:
                    nc.vector.tensor_mul(beta, ne_t, rinv)
                else:
                    nc.vector.scalar_tensor_tensor(
                        out=beta, in0=ne_t, scalar=float(beta_mult),
                        in1=rinv, op0=mybir.AluOpType.mult,
                        op1=mybir.AluOpType.mult)
                if beta_power != 1.0:
                    nc.scalar.activation(out=beta, in_=beta,
                                         func=mybir.ActivationFunctionType.Ln)
                    nc.vector.tensor_scalar_mul(beta, beta, float(beta_power))
                    nc.scalar.activation(out=beta, in_=beta,
                                         func=mybir.ActivationFunctionType.Exp)
                if general_beta:
                    nc.vector.tensor_scalar_min(beta, beta, 1.0)
                    nc.vector.tensor_scalar_max(beta, beta, 0.0)
                # ctx_t = ctx_prev + beta * (ce_t - ctx_prev)
                nc.vector.tensor_sub(dvec, ce_b[:, t, :], ctx_prev)
                nc.vector.scalar_tensor_tensor(
                    out=ctx_t, in0=dvec, scalar=beta, in1=ctx_prev,
                    op0=mybir.AluOpType.mult, op1=mybir.AluOpType.add)
                # ||ctx_t||
                nc.vector.scalar_tensor_tensor(
                    out=sqv, in0=ctx_t, scalar=1.0, in1=ctx_t,
                    op0=mybir.AluOpType.mult, op1=mybir.AluOpType.mult,
                    accum_out=nc2)
                nc.scalar.activation(out=ncur, in_=nc2,
                                     func=mybir.ActivationFunctionType.Sqrt)

            # ---- phase C: reshape ctx to (t b) rows via DRAM round-trip ----
            ctx_dram = dram_pool.tile([TB, C], F32)
            nc.sync.dma_start(out=ctx_dram.rearrange("(t b) c -> b t c", b=B),
                              in_=ctx_seq[:, 1:, :])

            # ---- phase D: ctxT2[(j,d), tb] = ctx[tb, d]  (j = 0,1) ----
            ctxT2 = mstage_pool.tile([128, TB], F32)
            for m0 in range(NM):
                ctx_tb = io_pool.tile([128, C], F32, name=f"ctx_tb_{m0}",
                                      tag="ctx_tb")
                nc.sync.dma_start(out=ctx_tb,
                                  in_=ctx_dram[m0 * 128:(m0 + 1) * 128, :])
                ctx_ps = ps_tr_pool.tile([C, 128], F32, name=f"ctx_ps_{m0}",
                                         tag="ctx_ps")
                nc.tensor.transpose(ctx_ps, ctx_tb, identity)
                nc.vector.tensor_copy(out=ctxT2[0:C, m0 * 128:(m0 + 1) * 128],
                                      in_=ctx_ps)
            # replicate ctx^T into the lower 64 partitions (cross-partition
            # moves need DMA, not DVE)
            nc.sync.dma_start(out=ctxT2[C:128, :], in_=ctxT2[0:C, :])

            # ---- phase E: mT[(c,d), tb] = aer * ctxT2 ----
            mT = mstage_pool.tile([128, NK, TB], w_dt)
            for k in range(NK):
                aer_k = io_pool.tile([128, TB], w_dt, name=f"aer_{k}",
                                     tag="aer")
                nc.sync.dma_start(out=aer_k,
                                  in_=aer_d.ap()[k * 128:(k + 1) * 128, :])
                nc.vector.tensor_mul(mT[:, k, :], aer_k, ctxT2)

            # ---- phase F: main matmul, W streamed once ----
            ps_tr_ctx.close()  # release transpose psum banks for accumulators
            ps_acc_ctx = ExitStack()
            ps_acc_pool = ps_acc_ctx.enter_context(
                tc.tile_pool(name="ps_acc", bufs=1, space="PSUM"))
            for vh in range(NVH):
                accs = {}
                for m0 in range(NM):
                    for v2 in range(VH // VT):
                        accs[(m0, v2)] = ps_acc_pool.tile(
                            [128, VT], F32, name=f"acc_{m0}_{v2}",
                            tag=f"acc_{m0}_{v2}")
                for k in range(NK):
                    w_k = w_pool.tile([128, VH], w_dt, name=f"w_{vh}_{k}",
                                      tag="w")
                    nc.sync.dma_start(
                        out=w_k,
                        in_=wt_d.ap()[k * 128:(k + 1) * 128,
                                      vh * VH:(vh + 1) * VH])
                    for m0 in range(NM):
                        lhsT = mm_cast(mT[:, k, m0 * 128:(m0 + 1) * 128])
                        for v2 in range(VH // VT):
                            nc.tensor.matmul(
                                accs[(m0, v2)],
                                lhsT,
                                mm_cast(w_k[:, v2 * VT:(v2 + 1) * VT]),
                                start=(k == 0), stop=(k == NK - 1))
                for m0 in range(NM):
                    for v2 in range(VH // VT):
                        o_sb = out_pool.tile([128, VT], F32,
                                             name=f"o_{vh}_{m0}_{v2}",
                                             tag="o")
                        nc.vector.tensor_copy(out=o_sb, in_=accs[(m0, v2)])
                        col = vh * VH + v2 * VT
                        nc.sync.dma_start(
                            out=out_d.ap()[m0 * 128:(m0 + 1) * 128,
                                           col:col + VT],
                            in_=o_sb)
            ps_acc_ctx.close()

    nc.finalize()
    return nc


def build_nc_v2(w_mode="bf16", beta_mult=1.0, beta_power=1.0,
                n_cores=N_CORES):
    """Pipelined builder: scan interleaved per 16-step chunk with ctx
    transpose + mT build + main matmuls (m0-outer, W half resident)."""
    from contextlib import ExitStack

    w_dt = _bir_wdt(w_mode)
    CH = T // NM               # 16 timesteps per chunk == one m0 tile

    nc = bacc.Bacc("TRN2", target_bir_lowering=False, debug=False,
                   num_devices=n_cores)

    ce_d = nc.dram_tensor("ce", [TB, C], F32, kind="ExternalInput")
    aer_d = nc.dram_tensor("aer", [K, TB], w_dt, kind="ExternalInput")
    wt_d = nc.dram_tensor("wt", [K, VS], w_dt, kind="ExternalInput")
    out_d = nc.dram_tensor("out", [TB, VS], F32, kind="ExternalOutput")

    with TileContext(nc) as tc:
        with (
            tc.tile_pool(name="const", bufs=1) as const_pool,
            tc.tile_pool(name="scan", bufs=1) as scan_pool,
            tc.tile_pool(name="mstage", bufs=1) as mstage_pool,
            tc.tile_pool(name="dram", bufs=1, space="DRAM") as dram_pool,
            tc.tile_pool(name="io", bufs=4) as io_pool,
            tc.tile_pool(name="wres", bufs=1) as w_pool,
            tc.tile_pool(name="ost", bufs=3) as out_pool,
            tc.tile_pool(name="ps_tr", bufs=2, space="PSUM") as ps_tr_pool,
            tc.tile_pool(name="ps_acc", bufs=1, space="PSUM") as ps_acc_pool,
        ):
            identity = const_pool.tile([128, 128], F32)
            make_identity(nc, identity)

            # W half 0: issue DMAs first so they stream during the scan.
            w_tiles = {}
            for k in range(NK):
                w_tiles[k] = w_pool.tile([128, VH], w_dt, name=f"w_0_{k}",
                                         tag=f"w{k}")
                nc.sync.dma_start(
                    out=w_tiles[k],
                    in_=wt_d.ap()[k * 128:(k + 1) * 128, 0:VH])

            # whole aer in one bulk DMA (no per-chunk descriptor traffic)
            aer_sb = mstage_pool.tile([128, NK, TB], w_dt)
            nc.sync.dma_start(
                out=aer_sb,
                in_=aer_d.ap().rearrange("(k p) tb -> p k tb", p=128))

            # scan state/scratch
            ce_b = scan_pool.tile([B, T, C], F32)
            ne = scan_pool.tile([B, T], F32)
            sq_ch = scan_pool.tile([B, CH * C], F32)
            ne2 = scan_pool.tile([B, CH, 1], F32)
            ctx_seq = scan_pool.tile([B, T + 1, C], F32)
            nc.vector.memset(ctx_seq[:, 0, :], 0.0)
            ncur = scan_pool.tile([B, 1], F32)
            nc.vector.memset(ncur, 0.0)
            ssum = scan_pool.tile([B, 1], F32)
            rinv = scan_pool.tile([B, 1], F32)
            beta = scan_pool.tile([B, 1], F32)
            nc2 = scan_pool.tile([B, 1], F32)
            dvec = scan_pool.tile([B, C], F32)
            sqv = scan_pool.tile([B, C], F32)

            ctx_dram = dram_pool.tile([TB, C], F32)
            ctxT2 = mstage_pool.tile([128, TB], F32)
            mT = mstage_pool.tile([128, NK, TB], w_dt)

            general_beta = (beta_mult != 1.0) or (beta_power != 1.0)

            for c_i in range(NM):
                t0 = c_i * CH
                # chunk inputs: ce slice + ne slice (off the critical chain)
                nc.scalar.dma_start(
                    out=ce_b[:, t0:t0 + CH, :],
                    in_=ce_d.ap().rearrange("(t b) c -> b t c", b=B)[
                        :, t0:t0 + CH, :])
                nc.vector.tensor_mul(
                    sq_ch.rearrange("b (t c) -> b t c", t=CH),
                    ce_b[:, t0:t0 + CH, :], ce_b[:, t0:t0 + CH, :])
                nc.vector.tensor_reduce(
                    out=ne2, in_=sq_ch.rearrange("b (t c) -> b t c", t=CH),
                    axis=mybir.AxisListType.X, op=mybir.AluOpType.add)
                nc.scalar.activation(
                    out=ne[:, t0:t0 + CH],
                    in_=ne2.rearrange("b t one -> b (t one)"),
                    func=mybir.ActivationFunctionType.Sqrt)

                for t in range(t0, t0 + CH):
                    ne_t = ne[:, t:t + 1]
                    ctx_prev = ctx_seq[:, t, :]
                    ctx_t = ctx_seq[:, t + 1, :]
                    nc.vector.tensor_add(ssum, ne_t, ncur)
                    if USE_DIVIDE and beta_mult == 1.0:
                        nc.vector.tensor_tensor(
                            out=beta, in0=ne_t, in1=ssum,
                            op=mybir.AluOpType.divide)
                    elif beta_mult == 1.0:
                        nc.vector.reciprocal(rinv, ssum)
                        nc.vector.tensor_mul(beta, ne_t, rinv)
                    else:
                        nc.vector.reciprocal(rinv, ssum)
                        nc.vector.scalar_tensor_tensor(
                            out=beta, in0=ne_t, scalar=float(beta_mult),
                            in1=rinv, op0=mybir.AluOpType.mult,
                            op1=mybir.AluOpType.mult)
                    if beta_power != 1.0:
                        nc.scalar.activation(
                            out=beta, in_=beta,
                            func=mybir.ActivationFunctionType.Ln)
                        nc.vector.tensor_scalar_mul(beta, beta,
                                                    float(beta_power))
                        nc.scalar.activation(
                            out=beta, in_=beta,
                            func=mybir.ActivationFunctionType.Exp)
                    if general_beta:
                        nc.vector.tensor_scalar_min(beta, beta, 1.0)
                        nc.vector.tensor_scalar_max(beta, beta, 0.0)
                    nc.vector.tensor_sub(dvec, ce_b[:, t, :], ctx_prev)
                    nc.vector.scalar_tensor_tensor(
                        out=ctx_t, in0=dvec, scalar=beta, in1=ctx_prev,
                        op0=mybir.AluOpType.mult, op1=mybir.AluOpType.add)
                    nc.vector.scalar_tensor_tensor(
                        out=sqv, in0=ctx_t, scalar=1.0, in1=ctx_t,
                        op0=mybir.AluOpType.mult, op1=mybir.AluOpType.mult,
                        accum_out=nc2)
                    nc.scalar.activation(
                        out=ncur, in_=nc2,
                        func=mybir.ActivationFunctionType.Sqrt)

                # chunk -> (t b) rows -> ctxT (PE transpose) -> ctxT2 cols
                m0 = c_i
                nc.scalar.dma_start(
                    out=ctx_dram.rearrange("(t b) c -> b t c", b=B)[
                        :, t0:t0 + CH, :],
                    in_=ctx_seq[:, t0 + 1:t0 + CH + 1, :])
                ctx_tb = io_pool.tile([128, C], F32, name=f"ctx_tb_{m0}",
                                      tag="ctx_tb")
                nc.scalar.dma_start(out=ctx_tb,
                                    in_=ctx_dram[m0 * 128:(m0 + 1) * 128, :])
                ctx_ps = ps_tr_pool.tile([C, 128], F32, name=f"ctx_ps_{m0}",
                                         tag="ctx_ps")
                nc.tensor.transpose(ctx_ps, ctx_tb, identity)
                nc.vector.tensor_copy(
                    out=ctxT2[0:C, m0 * 128:(m0 + 1) * 128], in_=ctx_ps)
                nc.scalar.dma_start(
                    out=ctxT2[C:128, m0 * 128:(m0 + 1) * 128],
                    in_=ctxT2[0:C, m0 * 128:(m0 + 1) * 128])

                # mT column block for this m0 — on GpSimd so the (in-order)
                # DVE stays free for the next chunk's scan steps
                for k in range(NK):
                    nc.gpsimd.tensor_mul(
                        mT[:, k, m0 * 128:(m0 + 1) * 128],
                        aer_sb[:, k, m0 * 128:(m0 + 1) * 128],
                        ctxT2[:, m0 * 128:(m0 + 1) * 128])

            # main matmuls: vh passes, m0-outer, k inner (W resident)
            for vh in range(NVH):
                if vh > 0:
                    for k in range(NK):
                        w_tiles[k] = w_pool.tile([128, VH], w_dt,
                                                 name=f"w_{vh}_{k}",
                                                 tag=f"w{k}")
                        nc.sync.dma_start(
                            out=w_tiles[k],
                            in_=wt_d.ap()[k * 128:(k + 1) * 128,
                                          vh * VH:(vh + 1) * VH])
                for m0 in range(NM):
                    accs = []
                    for v2 in range(VH // VT):
                        accs.append(ps_acc_pool.tile(
                            [128, VT], F32, name=f"acc_{vh}_{m0}_{v2}",
                            tag=f"acc_{m0 % 2}_{v2}"))
                    for k in range(NK):
                        lhsT = mT[:, k, m0 * 128:(m0 + 1) * 128]
                        for v2 in range(VH // VT):
                            nc.tensor.matmul(
                                accs[v2], lhsT,
                                w_tiles[k][:, v2 * VT:(v2 + 1) * VT],
                                start=(k == 0), stop=(k == NK - 1))
                    for v2 in range(VH // VT):
                        o_sb = out_pool.tile([128, VT], F32,
                                             name=f"o_{vh}_{m0}_{v2}",
                                             tag="o")
                        nc.vector.tensor_copy(out=o_sb, in_=accs[v2])
                        col = vh * VH + v2 * VT
                        nc.sync.dma_start(
                            out=out_d.ap()[m0 * 128:(m0 + 1) * 128,
                                           col:col + VT],
                            in_=o_sb)

    nc.finalize()
    return nc


def host_prep(tokens, emb_ctx, emb_act, W_a, w_mode=W_MODE):
    """Gather embeddings, build the replicated ae^T and the W shards."""
    wnp = _np_wdt(w_mode)
    tok = np.asarray(tokens).astype(np.int64).reshape(-1)  # [T*B], t-major
    ce = np.ascontiguousarray(np.asarray(emb_ctx, dtype=np.float32)[tok])
    ae = np.asarray(emb_act, dtype=np.float32)[tok]        # [TB, C]
    # aer[(c,d), tb] = ae[tb, c]
    aer = np.ascontiguousarray(
        np.repeat(ae.T, C, axis=0)).astype(wnp)            # [C*C, TB]
    W3 = np.asarray(W_a, dtype=np.float32).reshape(V, C, C)
    in_maps = []
    for s in range(N_CORES):
        ws = np.ascontiguousarray(
            W3[s * VS:(s + 1) * VS].transpose(1, 2, 0).reshape(K, VS)
        ).astype(wnp)                                      # [(c,d), v_local]
        in_maps.append({"ce": ce, "aer": aer, "wt": ws})
    return in_maps


_NC_CACHE = {}


def _get_nc(w_mode, beta_mult, beta_power):
    key = (w_mode, float(beta_mult), float(beta_power))
    if key not in _NC_CACHE:
        builder = build_nc_v2 if w_mode == "bf16" else build_nc
        _NC_CACHE[key] = builder(w_mode, *key[1:])
    return _NC_CACHE[key]


def install_ntff_shim():
    """Optional: register the axon NTFF profiling hook (for tracing)."""
    if "antenv.axon_hooks" in sys.modules:
        return
    m = types.ModuleType("antenv.axon_hooks")
    state = {"hook": None}
    m.get_axon_ntff_profile_hook = lambda: state["hook"]
    m.set_axon_ntff_profile_hook = lambda h: state.update(hook=h)
    sys.modules["antenv.axon_hooks"] = m
    try:
        from trn_agent_boot.trn_boot import _ntff_profile_via_ctypes

        state["hook"] = _ntff_profile_via_ctypes("/opt/axon/libaxon_pjrt.so")
    except Exception:
        pass


def run_hw(tokens, emb_ctx, emb_act, W_a, beta_mult, beta_power,
           w_mode=W_MODE, trace=False):
    if trace:
        install_ntff_shim()
    nc = _get_nc(w_mode, float(beta_mult), float(beta_power))
    in_maps = host_prep(tokens, emb_ctx, emb_act, W_a, w_mode)
    res = run_bass_kernel_spmd(nc, in_maps, core_ids=list(range(N_CORES)),
                               trace=trace)
    out = np.concatenate([res.results[s]["out"] for s in range(N_CORES)],
                         axis=1)                           # [TB, V]
    return out.reshape(T, B, V), res


def kernel(tokens, emb_ctx, emb_act, W_a, beta_mult, beta_power):
    out, _ = run_hw(tokens, emb_ctx, emb_act, W_a,
                    float(np.asarray(beta_mult)), float(np.asarray(beta_power)))
    return out
